# revision 39
# baseline (speedup 1.0000x reference)
"""Trainium2 Bass kernel for GQA attention prefill (nn_Attention).

Reference semantics (b=1, s=2048, dim=4096, 32 q heads, 8 kv heads, hd=128):
  xq = x @ wq.T ; xk = x @ wk.T ; xv = x @ wv.T
  xq, xk = rope(xq), rope(xk) ; xq, xk = rmsnorm(xq), rmsnorm(xk)
  o = softmax(q k^T / sqrt(hd) + mask) v          (grouped: 4 q heads / kv head)
  out = o @ wo.T

Sharding: tensor-parallel over heads on 8 cores — core c owns q heads
4c..4c+3 and kv head c; wo is sharded on its input dim; per-core partial
outputs are summed on the host.

Per-core pipeline (bf16 matmuls, fp32 accumulate):
  proj [s,e] -> rope+rmsnorm on sbuf bf16 -> PE-transpose q,k to [hd,s]
  scores^T[sk,sq] = kT.T @ qT ; exp on ACT (mask pre-scaled on host)
  denominator = ones-matmul over probsT chunks (PE, accumulated in psum)
  oT[hd,sq] += v.T @ probsT ; normalized via reciprocal+partition_broadcast
  out[s,d] += oT.T @ wo
Causality: fully-masked blocks skipped, diagonal blocks use real mask data.
"""

import math
import numpy as np
import ml_dtypes

import concourse.bass as bass
import concourse.tile as tile
from concourse import bacc, mybir
from concourse.bass import ts
from concourse.masks import make_identity
from concourse.bass_utils import run_bass_kernel_spmd

BF16 = mybir.dt.bfloat16
FP32 = mybir.dt.float32

N_CORES = 8
S = 2048          # sequence
D = 4096          # model dim
HD = 128          # head dim
HQ = 4            # q heads per core
E = HQ * HD       # q out dim per core (512)
T = S // 128      # 16 s-tiles of 128
CQ = S // 512     # 4 sq chunks of 512
KC = S // 128     # 16 sk chunks of 128
DC = 32           # d chunks of 128
EPS = 1e-5
INV_SQRT_HD = 1.0 / math.sqrt(HD)
SQRT_HD = math.sqrt(HD)

_CACHE = {}


def _pin_act_tables():
    """Keep every ACT function in one table set so no per-tile table
    reloads are emitted (Exp/Ln/Square/Copy all live in
    natural_log_exp_and_others)."""
    import functools
    import concourse.hw_specs as hw_specs
    import concourse.bass_interp as bass_interp
    orig = hw_specs.get_activation_tables

    @functools.cache
    def patched(module_arch):
        tabs = orig(module_arch)
        keep = "natural_log_exp_and_others"
        if keep not in tabs:
            return tabs
        E = mybir.ActivationFunctionType
        mine = {f for f in (getattr(E, n, None) for n in
                            ("Exp", "Ln", "Square", "Copy", "Identity"))
                if f is not None} & tabs[keep]
        # preserve set order/indices (act_func_set_id is positional); just
        # make `keep` the only set containing the functions this kernel uses
        return {name: (fns if name == keep else fns - mine)
                for name, fns in tabs.items()}

    bacc.get_activation_tables = patched
    bass_interp.get_activation_tables = patched


def build_bass(pin_tables=True, repeat=1):
    if pin_tables:
        _pin_act_tables()
    nc = bacc.Bacc("TRN2", target_bir_lowering=False, debug=False,
                   num_devices=N_CORES)

    xtt = nc.dram_tensor("xtt", [T, 128, DC, 128], BF16, kind="ExternalInput").ap()
    wqt = nc.dram_tensor("wqt", [128, DC, E], BF16, kind="ExternalInput").ap()
    wkvt = nc.dram_tensor("wkvt", [128, DC, 2 * HD], BF16, kind="ExternalInput").ap()
    wo = nc.dram_tensor("wo", [128, HQ, D], BF16, kind="ExternalInput").ap()
    mtd = nc.dram_tensor("mtd", [128, KC, 128], BF16, kind="ExternalInput").ap()
    cos = nc.dram_tensor("cos", [128, T, E], BF16, kind="ExternalInput").ap()
    sin = nc.dram_tensor("sin", [128, T, E], BF16, kind="ExternalInput").ap()
    out = nc.dram_tensor("out", [S, D], mybir.dt.float16, kind="ExternalOutput").ap()

    with tile.TileContext(nc) as tc:
        _emit(nc, tc, xtt, wqt, wkvt, wo, mtd, cos, sin, out, repeat=repeat)
    nc.compile()
    return nc


def _emit(nc, tc, xtt, wqt, wkvt, wo, mtd, cos, sin, out, repeat=1):
    from contextlib import ExitStack
    ctx = ExitStack()
    with ctx:
        res = ctx.enter_context(tc.tile_pool(name="res", bufs=1))
        xp = ctx.enter_context(tc.tile_pool(name="xp", bufs=2))
        fq = ctx.enter_context(tc.tile_pool(name="fq", bufs=2))
        sml = ctx.enter_context(tc.tile_pool(name="sml", bufs=2))
        pbuf = ctx.enter_context(tc.tile_pool(name="pbuf", bufs=10))
        accp = ctx.enter_context(tc.tile_pool(name="accp", bufs=3))
        stg = ctx.enter_context(tc.tile_pool(name="stg", bufs=6))
        psum = ctx.enter_context(tc.tile_pool(name="psum", bufs=1, space="PSUM"))

        # resident tensors
        wq_sb = res.tile([128, DC, E], BF16)
        wkv_sb = res.tile([128, DC, 2 * HD], BF16)
        wo_sb = res.tile([128, HQ, D], BF16)
        mtd_sb = res.tile([128, KC, 128], BF16)

        vsb = res.tile([128, T, HD], BF16)       # v, [s, hd] layout
        qkT = res.tile([128, 5, T, 128], BF16)   # slots 0-3: qT heads, 4: kT
        oT = res.tile([128, HQ, T, 128], BF16)   # o^T per head: [hd, s]

        ident = res.tile([128, 128], BF16)
        make_identity(nc, ident[:])
        ones = res.tile([128, 1], BF16)
        nc.vector.memset(ones[:], 1.0)
        epsb = res.tile([128, 1], FP32)
        nc.vector.memset(epsb[:], EPS)

        AF = mybir.ActivationFunctionType
        MUL = mybir.AluOpType.mult
        ADD = mybir.AluOpType.add

        # psum budget (8 banks): pa 2 {psq,pskv} + aux 2 {ptr,pd,pw} +
        # ps 3 (B scores, depth-2 pipeline) + po 1 (PV accumulator)
        PA = dict(tag="mix", bufs=4)
        AUX = dict(tag="aux", bufs=2)
        PS = dict(tag="mix", bufs=4)
        PO = dict(tag="po", bufs=2)

        loop_ctx = tc.For_i(0, repeat, 1) if repeat > 1 else None
        if loop_ctx is not None:
            ctx.enter_context(loop_ctx)

        # ---------------- Phase A: proj + rope + rmsnorm + transposes
        # Software-pipelined: PE stream is [proj(m), transposes(m-1), ...] so
        # the DVE/ACT chain of tile m runs under proj(m+1)'s matmuls.
        def a_proj(m, first):
            xc = xp.tile([128, DC, 128], BF16, tag="xc", name=f"xc{m}")
            if m == 0:
                # first accumulation step's operands lead the DMA queue
                nc.sync.dma_start(xc[:, 0:8, :], xtt[m][:, 0:8, :])
                nc.sync.dma_start(wq_sb[:, 0:8, :], wqt[:, 0:8, :])
                for g in range(8, DC, 8):
                    nc.sync.dma_start(xc[:, g:g + 8, :], xtt[m][:, g:g + 8, :])
                    nc.sync.dma_start(wq_sb[:, g:g + 8, :], wqt[:, g:g + 8, :])
                for g in range(0, DC, 8):
                    nc.sync.dma_start(wkv_sb[:, g:g + 8, :], wkvt[:, g:g + 8, :])
            elif m < 2:
                for g in range(0, DC, 8):
                    nc.sync.dma_start(xc[:, g:g + 8, :], xtt[m][:, g:g + 8, :])
            else:
                nc.sync.dma_start(xc[:], xtt[m])
            cs = fq.tile([128, E], BF16, tag="cos", name=f"cs{m}")
            nc.sync.dma_start(cs[:], cos[:, m, :])
            sn = fq.tile([128, E], BF16, tag="sin", name=f"sn{m}")
            nc.sync.dma_start(sn[:], sin[:, m, :])
            if m == 0:
                nc.sync.dma_start(mtd_sb[:], mtd[:])
            if m == 2:
                nc.sync.dma_start(wo_sb[:], wo[:])

            psq = psum.tile([128, E], FP32, name=f"psq{m}", **PA)
            pskv = psum.tile([128, 2 * HD], FP32, name=f"pskv{m}", **PA)
            for c in range(DC):
                nc.tensor.matmul(psq[:], xc[:, c, :], wq_sb[:, c, :],
                                 start=(c == 0), stop=(c == DC - 1))
            for c in range(DC):
                nc.tensor.matmul(pskv[:], xc[:, c, :], wkv_sb[:, c, :],
                                 start=(c == 0), stop=(c == DC - 1))

            # evacuate psum to sbuf bf16 (ACT), then rope on DVE in 2x mode
            qsb = sml.tile([128, E], BF16, tag="qsb", name=f"qsb{m}")
            nc.scalar.copy(qsb[:], psq[:])
            kvsb = sml.tile([128, 2 * HD], BF16, tag="kvsb", name=f"kvsb{m}")
            nc.scalar.copy(kvsb[:], pskv[:])
            nc.vector.tensor_copy(vsb[:, m, :], kvsb[:, HD:2 * HD])

            # rope(q): qro = q*cos + swap(q)*sin_signed
            tco = sml.tile([128, E], BF16, tag="tco", name=f"tco{m}")
            nc.vector.tensor_tensor(tco[:], qsb[:], cs[:], op=MUL)
            tro = sml.tile([128, E], BF16, tag="tro", name=f"tro{m}")
            q3 = qsb[:].rearrange("p (x two) -> p x two", two=2)
            t3 = tro[:].rearrange("p (x two) -> p x two", two=2)
            nc.vector.tensor_copy(t3[:, :, 0], q3[:, :, 1])
            nc.vector.tensor_copy(t3[:, :, 1], q3[:, :, 0])
            qro = sml.tile([128, E], BF16, tag="qro", name=f"qro{m}")
            nc.vector.tensor_tensor(tro[:], tro[:], sn[:], op=MUL)
            nc.vector.tensor_tensor(qro[:], tco[:], tro[:], op=ADD)

            # rope(k)
            tck = sml.tile([128, E], BF16, tag="tco", name=f"tck{m}")
            nc.vector.tensor_tensor(tck[:, 0:HD], kvsb[:, 0:HD], cs[:, 0:HD], op=MUL)
            trk = sml.tile([128, E], BF16, tag="tro", name=f"trk{m}")
            k3 = kvsb[:, 0:HD].rearrange("p (x two) -> p x two", two=2)
            r3 = trk[:, 0:HD].rearrange("p (x two) -> p x two", two=2)
            nc.vector.tensor_copy(r3[:, :, 0], k3[:, :, 1])
            nc.vector.tensor_copy(r3[:, :, 1], k3[:, :, 0])
            kro = sml.tile([128, E], BF16, tag="qro", name=f"kro{m}")
            nc.vector.tensor_tensor(trk[:, 0:HD], trk[:, 0:HD], sn[:, 0:HD], op=MUL)
            nc.vector.tensor_tensor(kro[:, 0:HD], tck[:, 0:HD], trk[:, 0:HD], op=ADD)

            # rmsnorm: rinv = exp(-0.5*ln(mean(t^2)+eps)), all in one ACT set
            sqs = sml.tile([128, 5], FP32, tag="sqs", name=f"sqs{m}")
            scr = sml.tile([128, HD], FP32, tag="scr", name=f"scr{m}")
            for h in range(HQ):
                nc.scalar.activation(scr[:], qro[:, ts(h, HD)], AF.Square,
                                     accum_out=sqs[:, h:h + 1])
            nc.scalar.activation(scr[:], kro[:, 0:HD], AF.Square,
                                 accum_out=sqs[:, 4:5])
            rin = sml.tile([128, 5], FP32, tag="rin", name=f"rin{m}")
            nc.scalar.activation(rin[:], sqs[:], AF.Ln, scale=1.0 / HD, bias=epsb[:])
            nc.scalar.activation(rin[:], rin[:], AF.Exp, scale=-0.5)

            qnt = sml.tile([128, E], BF16, tag="qnt", bufs=3, name=f"qnt{m}")
            knt = sml.tile([128, HD], BF16, tag="knt", bufs=3, name=f"knt{m}")
            for h in range(HQ):
                nc.scalar.mul(qnt[:, ts(h, HD)], qro[:, ts(h, HD)], rin[:, h:h + 1])
            nc.scalar.mul(knt[:], kro[:, 0:HD], rin[:, 4:5])
            return qnt, knt

        def a_post(m, qnt, knt):
            # transposes packed into one psum bank, single evac
            ptr = psum.tile([128, 5, 128], BF16, name=f"ptr{m}", **AUX)
            for h in range(HQ):
                nc.tensor.transpose(ptr[:, h, :], qnt[:, ts(h, HD)], ident[:])
            nc.tensor.transpose(ptr[:, 4, :], knt[:], ident[:])
            nc.scalar.copy(qkT[:, :, m, :], ptr[:])

        prev = None
        for m in range(T):
            qk = a_proj(m, first=(m == 0))
            if prev is not None:
                a_post(m - 1, *prev)
            prev = qk
        a_post(T - 1, *prev)

        # ---------------- Phase B: attention, PE-stream pipelined depth 2:
        # [QK(0), QK(1), QK(2), PV(0), QK(3), PV(1), ...]
        for cq in range(CQ):
            nkc = 4 * cq + 4
            for h in range(HQ):
                po = psum.tile([128, 512], FP32, name=f"po{cq}_{h}", **PO)
                pd = psum.tile([1, 512], FP32, name=f"pdn{cq}_{h}", **AUX)

                def qk_exp(kc):
                    # columns below j0 are fully masked (causal): skip them
                    j0 = max(0, 128 * (kc - 4 * cq))
                    ps = psum.tile([128, 512], FP32, name=f"sc{cq}_{h}_{kc}", **PS)
                    d0 = j0 // 128
                    nc.tensor.matmul(ps[:, j0:], qkT[:, 4, kc, :],
                                     qkT[:, h, 4 * cq + d0:4 * cq + 4, :],
                                     start=True, stop=True)
                    pt = pbuf.tile([128, 512], BF16, tag="pt",
                                   name=f"pt{cq}_{h}_{kc}")
                    nc.scalar.activation(pt[:, j0:], ps[:, j0:], AF.Exp,
                                         scale=INV_SQRT_HD)
                    if kc >= 4 * cq:
                        # partially masked triangle block: 0/1 mask multiply
                        j1 = min(j0 + 128, 512)
                        nc.vector.tensor_tensor(pt[:, j0:j1], pt[:, j0:j1],
                                                mtd_sb[:, kc, 0:j1 - j0], op=MUL)
                    return pt

                def pv(kc, pt):
                    j0 = max(0, 128 * (kc - 4 * cq))
                    nc.tensor.matmul(po[:, j0:], vsb[:, kc, :], pt[:, j0:],
                                     start=(kc == 0), stop=(kc == nkc - 1))
                    nc.tensor.matmul(pd[:, j0:], ones[:], pt[:, j0:],
                                     start=(kc == 0), stop=(kc == nkc - 1))

                depth = 3
                pts = [qk_exp(kc) for kc in range(min(depth, nkc))]
                for kc in range(nkc):
                    if kc + depth < nkc:
                        pts.append(qk_exp(kc + depth))
                    pv(kc, pts[kc])

                rec = accp.tile([1, 512], FP32, tag="rec", name=f"rec{cq}_{h}")
                nc.vector.reciprocal(rec[:], pd[:])
                rb = accp.tile([128, 512], FP32, tag="rb", name=f"rb{cq}_{h}")
                nc.gpsimd.partition_broadcast(rb[:], rec[:])
                nc.vector.tensor_tensor(oT[:, h, ts(cq, 4), :], po[:], rb[:],
                                        op=MUL)

            # ---------------- Phase C: output projection for this cq block
            for m in range(4 * cq, 4 * cq + 4):
                for dc in range(8):
                    pw = psum.tile([128, 512], FP32, name=f"pw{m}_{dc}", **PS)
                    for j in range(HQ):
                        nc.tensor.matmul(pw[:], oT[:, j, m, :],
                                         wo_sb[:, j, ts(dc, 512)],
                                         start=(j == 0), stop=(j == HQ - 1))
                    so = stg.tile([128, 512], mybir.dt.float16, tag="so", name=f"so{m}_{dc}")
                    nc.vector.tensor_copy(so[:], pw[:])
                    nc.sync.dma_start(out[ts(m, 128), ts(dc, 512)], so[:])


def _prep_inputs(x, wq, wk, wv, wo, freqs_cis, mask):
    """Host-side shard + retile. Returns list of per-core input dicts."""
    bf = ml_dtypes.bfloat16
    x2 = np.asarray(x, dtype=np.float32).reshape(S, D)
    # xtt[m, p, c, s'] = x[128m+s', 128c+p]
    xtt = np.ascontiguousarray(
        x2.reshape(T, 128, DC, 128).transpose(0, 3, 2, 1)).astype(bf)

    fr = np.asarray(freqs_cis, dtype=np.float32)[..., 0]   # [S, 64]
    fi = np.asarray(freqs_cis, dtype=np.float32)[..., 1]
    cos_il = np.repeat(fr, 2, axis=1)                       # [S, 128]
    sin_il = np.repeat(fi, 2, axis=1)
    sin_il[:, 0::2] *= -1.0                                 # signed for swap-form
    cos_rep = np.tile(cos_il, (1, HQ))                      # [S, 512]
    sin_rep = np.tile(sin_il, (1, HQ))
    cos_t = np.ascontiguousarray(
        cos_rep.reshape(T, 128, E).transpose(1, 0, 2)).astype(bf)
    sin_t = np.ascontiguousarray(
        sin_rep.reshape(T, 128, E).transpose(1, 0, 2)).astype(bf)

    m2 = np.asarray(mask, dtype=np.float32)
    # per sk-chunk kc: the 128-wide partially-masked column block of the
    # transposed mask, as 0/1 visibility. mtd_t[p, kc, j] for global key
    # row 128*kc+p, query col 512*cq + 128*(kc-4*cq) + j.
    mtd_t = np.zeros((128, KC, 128), dtype=np.float32)
    for kc in range(KC):
        cq = kc // 4
        j0 = 128 * (kc - 4 * cq)
        qlo = 512 * cq + j0
        blk = m2[qlo:qlo + 128, 128 * kc:128 * (kc + 1)].T  # [sk 128, sq 128]
        mtd_t[:, kc, :] = (blk > -1e29)
    mtd_t = np.ascontiguousarray(mtd_t).astype(bf)

    wqf = np.asarray(wq, dtype=np.float32)
    wkf = np.asarray(wk, dtype=np.float32)
    wvf = np.asarray(wv, dtype=np.float32)
    wof = np.asarray(wo, dtype=np.float32)

    in_maps = []
    for c in range(N_CORES):
        wq_c = wqf[E * c:E * (c + 1), :]                    # [512, D]
        wqt = np.ascontiguousarray(
            wq_c.T.reshape(DC, 128, E).transpose(1, 0, 2)).astype(bf)
        wk_c = wkf[HD * c:HD * (c + 1), :]                  # [128, D]
        wv_c = wvf[HD * c:HD * (c + 1), :]
        wkv_c = np.concatenate([wk_c, wv_c], axis=0)        # [256, D]
        wkvt = np.ascontiguousarray(
            wkv_c.T.reshape(DC, 128, 2 * HD).transpose(1, 0, 2)).astype(bf)
        wo_c = wof[:, E * c:E * (c + 1)].T                  # [512 e, D]
        wo_t = np.ascontiguousarray(
            wo_c.reshape(HQ, 128, D).transpose(1, 0, 2)).astype(bf)
        in_maps.append({
            "xtt": xtt, "wqt": wqt, "wkvt": wkvt, "wo": wo_t,
            "mtd": mtd_t, "cos": cos_t, "sin": sin_t,
        })
    return in_maps


def kernel(x, wq, wk, wv, wo, freqs_cis, mask, start_pos=0):
    if "nc" not in _CACHE:
        _CACHE["nc"] = build_bass()
    nc = _CACHE["nc"]
    in_maps = _prep_inputs(x, wq, wk, wv, wo, freqs_cis, mask)
    res = run_bass_kernel_spmd(nc, in_maps, list(range(N_CORES)))
    total = np.zeros((S, D), dtype=np.float32)
    for c in range(N_CORES):
        total += res.results[c]["out"].astype(np.float32)
    return total.reshape(1, S, D)


# revision 45
# speedup vs baseline: 1.1943x; 1.1943x over previous
"""Trainium2 Bass kernel for GQA attention prefill (nn_Attention).

Reference semantics (b=1, s=2048, dim=4096, 32 q heads, 8 kv heads, hd=128):
  xq = x @ wq.T ; xk = x @ wk.T ; xv = x @ wv.T
  xq, xk = rope(xq), rope(xk) ; xq, xk = rmsnorm(xq), rmsnorm(xk)
  o = softmax(q k^T / sqrt(hd) + mask) v          (grouped: 4 q heads / kv head)
  out = o @ wo.T

Sharding: tensor-parallel over heads on 8 cores — core c owns q heads
4c..4c+3 and kv head c; wo is sharded on its input dim; per-core partial
outputs are summed on the host.

Per-core pipeline (bf16 matmuls, fp32 accumulate):
  proj [s,e] -> rope+rmsnorm on sbuf bf16 -> PE-transpose q,k to [hd,s]
  scores^T[sk,sq] = kT.T @ qT ; exp on ACT (mask pre-scaled on host)
  denominator = ones-matmul over probsT chunks (PE, accumulated in psum)
  oT[hd,sq] += v.T @ probsT ; normalized via reciprocal+partition_broadcast
  out[s,d] += oT.T @ wo
Causality: fully-masked blocks skipped, diagonal blocks use real mask data.
"""

import math
import numpy as np
import ml_dtypes

import concourse.bass as bass
import concourse.tile as tile
from concourse import bacc, mybir
from concourse.bass import ts
from concourse.masks import make_identity
from concourse.bass_utils import run_bass_kernel_spmd

BF16 = mybir.dt.bfloat16
FP32 = mybir.dt.float32

N_CORES = 8
S = 2048          # sequence
D = 4096          # model dim
HD = 128          # head dim
HQ = 4            # q heads per core
E = HQ * HD       # q out dim per core (512)
T = S // 128      # 16 s-tiles of 128
CQ = S // 512     # 4 sq chunks of 512
KC = S // 128     # 16 sk chunks of 128
DC = 32           # d chunks of 128
EPS = 1e-5
INV_SQRT_HD = 1.0 / math.sqrt(HD)
SQRT_HD = math.sqrt(HD)

_CACHE = {}


def _pin_act_tables():
    """Keep every ACT function in one table set so no per-tile table
    reloads are emitted (Exp/Ln/Square/Copy all live in
    natural_log_exp_and_others)."""
    import functools
    import concourse.hw_specs as hw_specs
    import concourse.bass_interp as bass_interp
    orig = hw_specs.get_activation_tables

    @functools.cache
    def patched(module_arch):
        tabs = orig(module_arch)
        keep = "natural_log_exp_and_others"
        if keep not in tabs:
            return tabs
        E = mybir.ActivationFunctionType
        mine = {f for f in (getattr(E, n, None) for n in
                            ("Exp", "Ln", "Square", "Copy", "Identity"))
                if f is not None} & tabs[keep]
        # preserve set order/indices (act_func_set_id is positional); just
        # make `keep` the only set containing the functions this kernel uses
        return {name: (fns if name == keep else fns - mine)
                for name, fns in tabs.items()}

    bacc.get_activation_tables = patched
    bass_interp.get_activation_tables = patched


def build_bass(pin_tables=True, repeat=1):
    if pin_tables:
        _pin_act_tables()
    nc = bacc.Bacc("TRN2", target_bir_lowering=False, debug=False,
                   num_devices=N_CORES)

    xtt = nc.dram_tensor("xtt", [T, 128, DC, 128], BF16, kind="ExternalInput").ap()
    wqt = nc.dram_tensor("wqt", [128, DC, E], BF16, kind="ExternalInput").ap()
    wkvt = nc.dram_tensor("wkvt", [128, DC, 2 * HD], BF16, kind="ExternalInput").ap()
    wo = nc.dram_tensor("wo", [128, HQ, D], BF16, kind="ExternalInput").ap()
    mtd = nc.dram_tensor("mtd", [128, KC, 128], BF16, kind="ExternalInput").ap()
    cos = nc.dram_tensor("cos", [128, T, E], BF16, kind="ExternalInput").ap()
    sin = nc.dram_tensor("sin", [128, T, E], BF16, kind="ExternalInput").ap()
    out = nc.dram_tensor("out", [S, D], mybir.dt.float16, kind="ExternalOutput").ap()

    with tile.TileContext(nc) as tc:
        _emit(nc, tc, xtt, wqt, wkvt, wo, mtd, cos, sin, out, repeat=repeat)
    nc.compile()
    return nc


def _emit(nc, tc, xtt, wqt, wkvt, wo, mtd, cos, sin, out, repeat=1):
    from contextlib import ExitStack
    ctx = ExitStack()
    with ctx:
        res = ctx.enter_context(tc.tile_pool(name="res", bufs=1))
        xp = ctx.enter_context(tc.tile_pool(name="xp", bufs=2))
        fq = ctx.enter_context(tc.tile_pool(name="fq", bufs=2))
        sml = ctx.enter_context(tc.tile_pool(name="sml", bufs=2))
        pbuf = ctx.enter_context(tc.tile_pool(name="pbuf", bufs=10))
        accp = ctx.enter_context(tc.tile_pool(name="accp", bufs=3))
        stg = ctx.enter_context(tc.tile_pool(name="stg", bufs=6))
        psum = ctx.enter_context(tc.tile_pool(name="psum", bufs=1, space="PSUM"))

        # resident tensors
        wq_sb = res.tile([128, DC, E], BF16)
        wkv_sb = res.tile([128, DC, 2 * HD], BF16)
        wo_sb = res.tile([128, HQ, D], BF16)
        mtd_sb = res.tile([128, KC, 128], BF16)

        vsb = res.tile([128, T, HD], BF16)       # v, [s, hd] layout
        qkT = res.tile([128, 5, T, 128], BF16)   # slots 0-3: qT heads, 4: kT
        oT = res.tile([128, HQ, T, 128], BF16)   # o^T per head: [hd, s]

        ident = res.tile([128, 128], BF16)
        make_identity(nc, ident[:])
        ones = res.tile([128, 1], BF16)
        nc.vector.memset(ones[:], 1.0)
        epsb = res.tile([128, 1], FP32)
        nc.vector.memset(epsb[:], EPS)

        AF = mybir.ActivationFunctionType
        MUL = mybir.AluOpType.mult
        ADD = mybir.AluOpType.add

        # psum budget (8 banks): pa 2 {psq,pskv} + aux 2 {ptr,pd,pw} +
        # ps 3 (B scores, depth-2 pipeline) + po 1 (PV accumulator)
        PA = dict(tag="mix", bufs=4)
        AUX = dict(tag="aux", bufs=2)
        PS = dict(tag="mix", bufs=4)
        PO = dict(tag="po", bufs=2)

        loop_ctx = tc.For_i(0, repeat, 1) if repeat > 1 else None
        if loop_ctx is not None:
            ctx.enter_context(loop_ctx)

        # ---------------- Phase A: proj + rope + rmsnorm + transposes
        # Software-pipelined: PE stream is [proj(m), transposes(m-1), ...] so
        # the DVE/ACT chain of tile m runs under proj(m+1)'s matmuls.
        def a_proj(m, first):
            xc = xp.tile([128, DC, 128], BF16, tag="xc", name=f"xc{m}")
            if m == 0:
                # first accumulation step's operands lead the DMA queue
                nc.sync.dma_start(xc[:, 0:8, :], xtt[m][:, 0:8, :])
                nc.sync.dma_start(wq_sb[:, 0:8, :], wqt[:, 0:8, :])
                for g in range(8, DC, 8):
                    nc.sync.dma_start(xc[:, g:g + 8, :], xtt[m][:, g:g + 8, :])
                    nc.sync.dma_start(wq_sb[:, g:g + 8, :], wqt[:, g:g + 8, :])
                for g in range(0, DC, 8):
                    nc.sync.dma_start(wkv_sb[:, g:g + 8, :], wkvt[:, g:g + 8, :])
            elif m < 2:
                for g in range(0, DC, 8):
                    nc.sync.dma_start(xc[:, g:g + 8, :], xtt[m][:, g:g + 8, :])
            else:
                nc.sync.dma_start(xc[:], xtt[m])
            cs = fq.tile([128, E], BF16, tag="cos", name=f"cs{m}")
            nc.sync.dma_start(cs[:], cos[:, m, :])
            sn = fq.tile([128, E], BF16, tag="sin", name=f"sn{m}")
            nc.sync.dma_start(sn[:], sin[:, m, :])
            if m == 0:
                nc.sync.dma_start(mtd_sb[:], mtd[:])
            if m == 2:
                nc.sync.dma_start(wo_sb[:], wo[:])

            psq = psum.tile([128, E], FP32, name=f"psq{m}", **PA)
            pskv = psum.tile([128, 2 * HD], FP32, name=f"pskv{m}", **PA)
            for c in range(DC):
                nc.tensor.matmul(psq[:], xc[:, c, :], wq_sb[:, c, :],
                                 start=(c == 0), stop=(c == DC - 1))
            for c in range(DC):
                nc.tensor.matmul(pskv[:], xc[:, c, :], wkv_sb[:, c, :],
                                 start=(c == 0), stop=(c == DC - 1))

            # evacuate psum to sbuf bf16 (ACT), then rope on DVE in 2x mode
            qsb = sml.tile([128, E], BF16, tag="qsb", name=f"qsb{m}")
            nc.scalar.copy(qsb[:], psq[:])
            kvsb = sml.tile([128, 2 * HD], BF16, tag="kvsb", name=f"kvsb{m}")
            nc.scalar.copy(kvsb[:], pskv[:])
            nc.vector.tensor_copy(vsb[:, m, :], kvsb[:, HD:2 * HD])

            # rope(q): qro = q*cos + swap(q)*sin_signed
            tco = sml.tile([128, E], BF16, tag="tco", name=f"tco{m}")
            nc.vector.tensor_tensor(tco[:], qsb[:], cs[:], op=MUL)
            tro = sml.tile([128, E], BF16, tag="tro", name=f"tro{m}")
            q3 = qsb[:].rearrange("p (x two) -> p x two", two=2)
            t3 = tro[:].rearrange("p (x two) -> p x two", two=2)
            nc.vector.tensor_copy(t3[:, :, 0], q3[:, :, 1])
            nc.vector.tensor_copy(t3[:, :, 1], q3[:, :, 0])
            qro = sml.tile([128, E], BF16, tag="qro", name=f"qro{m}")
            nc.vector.tensor_tensor(tro[:], tro[:], sn[:], op=MUL)
            nc.vector.tensor_tensor(qro[:], tco[:], tro[:], op=ADD)

            # rope(k)
            tck = sml.tile([128, E], BF16, tag="tco", name=f"tck{m}")
            nc.vector.tensor_tensor(tck[:, 0:HD], kvsb[:, 0:HD], cs[:, 0:HD], op=MUL)
            trk = sml.tile([128, E], BF16, tag="tro", name=f"trk{m}")
            k3 = kvsb[:, 0:HD].rearrange("p (x two) -> p x two", two=2)
            r3 = trk[:, 0:HD].rearrange("p (x two) -> p x two", two=2)
            nc.vector.tensor_copy(r3[:, :, 0], k3[:, :, 1])
            nc.vector.tensor_copy(r3[:, :, 1], k3[:, :, 0])
            kro = sml.tile([128, E], BF16, tag="qro", name=f"kro{m}")
            nc.vector.tensor_tensor(trk[:, 0:HD], trk[:, 0:HD], sn[:, 0:HD], op=MUL)
            nc.vector.tensor_tensor(kro[:, 0:HD], tck[:, 0:HD], trk[:, 0:HD], op=ADD)

            # rmsnorm: rinv = exp(-0.5*ln(mean(t^2)+eps)), all in one ACT set
            sqs = sml.tile([128, 5], FP32, tag="sqs", name=f"sqs{m}")
            scr = sml.tile([128, HD], FP32, tag="scr", name=f"scr{m}")
            for h in range(HQ):
                nc.scalar.activation(scr[:], qro[:, ts(h, HD)], AF.Square,
                                     accum_out=sqs[:, h:h + 1])
            nc.scalar.activation(scr[:], kro[:, 0:HD], AF.Square,
                                 accum_out=sqs[:, 4:5])
            rin = sml.tile([128, 5], FP32, tag="rin", name=f"rin{m}")
            nc.scalar.activation(rin[:], sqs[:], AF.Ln, scale=1.0 / HD, bias=epsb[:])
            nc.scalar.activation(rin[:], rin[:], AF.Exp, scale=-0.5)

            qnt = sml.tile([128, E], BF16, tag="qnt", bufs=3, name=f"qnt{m}")
            knt = sml.tile([128, HD], BF16, tag="knt", bufs=3, name=f"knt{m}")
            for h in range(HQ):
                nc.scalar.mul(qnt[:, ts(h, HD)], qro[:, ts(h, HD)], rin[:, h:h + 1])
            nc.scalar.mul(knt[:], kro[:, 0:HD], rin[:, 4:5])
            return qnt, knt

        def a_post(m, qnt, knt):
            # transposes packed into one psum bank, single evac
            ptr = psum.tile([128, 5, 128], BF16, name=f"ptr{m}", **AUX)
            for h in range(HQ):
                nc.tensor.transpose(ptr[:, h, :], qnt[:, ts(h, HD)], ident[:])
            nc.tensor.transpose(ptr[:, 4, :], knt[:], ident[:])
            nc.scalar.copy(qkT[:, :, m, :], ptr[:])

        prev = None
        for m in range(T):
            qk = a_proj(m, first=(m == 0))
            if prev is not None:
                a_post(m - 1, *prev)
            prev = qk
        a_post(T - 1, *prev)

        # ---------------- Phase B: attention, PE-stream pipelined depth 2:
        # [QK(0), QK(1), QK(2), PV(0), QK(3), PV(1), ...]
        for cq in range(CQ):
            nkc = 4 * cq + 4
            for h in range(HQ):
                po = psum.tile([128, 512], FP32, name=f"po{cq}_{h}", **PO)
                pd = psum.tile([1, 512], FP32, name=f"pdn{cq}_{h}", **AUX)

                def qk_exp(kc):
                    # columns below j0 are fully masked (causal): skip them
                    j0 = max(0, 128 * (kc - 4 * cq))
                    ps = psum.tile([128, 512], FP32, name=f"sc{cq}_{h}_{kc}", **PS)
                    d0 = j0 // 128
                    nc.tensor.matmul(ps[:, j0:], qkT[:, 4, kc, :],
                                     qkT[:, h, 4 * cq + d0:4 * cq + 4, :],
                                     start=True, stop=True)
                    pt = pbuf.tile([128, 512], BF16, tag="pt",
                                   name=f"pt{cq}_{h}_{kc}")
                    nc.scalar.activation(pt[:, j0:], ps[:, j0:], AF.Exp,
                                         scale=INV_SQRT_HD)
                    if kc >= 4 * cq:
                        # partially masked triangle block: 0/1 mask multiply
                        j1 = min(j0 + 128, 512)
                        nc.vector.tensor_tensor(pt[:, j0:j1], pt[:, j0:j1],
                                                mtd_sb[:, kc, 0:j1 - j0], op=MUL)
                    return pt

                def pv(kc, pt):
                    j0 = max(0, 128 * (kc - 4 * cq))
                    nc.tensor.matmul(po[:, j0:], vsb[:, kc, :], pt[:, j0:],
                                     start=(kc == 0), stop=(kc == nkc - 1))
                    nc.tensor.matmul(pd[:, j0:], ones[:], pt[:, j0:],
                                     start=(kc == 0), stop=(kc == nkc - 1))

                depth = 3
                pts = [qk_exp(kc) for kc in range(min(depth, nkc))]
                for kc in range(nkc):
                    if kc + depth < nkc:
                        pts.append(qk_exp(kc + depth))
                    pv(kc, pts[kc])

                rec = accp.tile([1, 512], FP32, tag="rec", name=f"rec{cq}_{h}")
                nc.vector.reciprocal(rec[:], pd[:])
                rb = accp.tile([128, 512], FP32, tag="rb", name=f"rb{cq}_{h}")
                nc.gpsimd.partition_broadcast(rb[:], rec[:])
                nc.vector.tensor_tensor(oT[:, h, ts(cq, 4), :], po[:], rb[:],
                                        op=MUL)

            # ---------------- Phase C: output projection for this cq block
            for m in range(4 * cq, 4 * cq + 4):
                for dc in range(8):
                    pw = psum.tile([128, 512], FP32, name=f"pw{m}_{dc}", **PS)
                    for j in range(HQ):
                        nc.tensor.matmul(pw[:], oT[:, j, m, :],
                                         wo_sb[:, j, ts(dc, 512)],
                                         start=(j == 0), stop=(j == HQ - 1))
                    so = stg.tile([128, 512], mybir.dt.float16, tag="so", name=f"so{m}_{dc}")
                    nc.vector.tensor_copy(so[:], pw[:])
                    nc.sync.dma_start(out[ts(m, 128), ts(dc, 512)], so[:])


def _prep_inputs(x, wq, wk, wv, wo, freqs_cis, mask):
    """Host-side shard + retile. Returns list of per-core input dicts."""
    bf = ml_dtypes.bfloat16
    x2 = np.asarray(x, dtype=np.float32).reshape(S, D)
    # xtt[m, p, c, s'] = x[128m+s', 128c+p]
    xtt = np.ascontiguousarray(
        x2.reshape(T, 128, DC, 128).transpose(0, 3, 2, 1)).astype(bf)

    fr = np.asarray(freqs_cis, dtype=np.float32)[..., 0]   # [S, 64]
    fi = np.asarray(freqs_cis, dtype=np.float32)[..., 1]
    cos_il = np.repeat(fr, 2, axis=1)                       # [S, 128]
    sin_il = np.repeat(fi, 2, axis=1)
    sin_il[:, 0::2] *= -1.0                                 # signed for swap-form
    cos_rep = np.tile(cos_il, (1, HQ))                      # [S, 512]
    sin_rep = np.tile(sin_il, (1, HQ))
    cos_t = np.ascontiguousarray(
        cos_rep.reshape(T, 128, E).transpose(1, 0, 2)).astype(bf)
    sin_t = np.ascontiguousarray(
        sin_rep.reshape(T, 128, E).transpose(1, 0, 2)).astype(bf)

    m2 = np.asarray(mask, dtype=np.float32)
    # per sk-chunk kc: the 128-wide partially-masked column block of the
    # transposed mask, as 0/1 visibility. mtd_t[p, kc, j] for global key
    # row 128*kc+p, query col 512*cq + 128*(kc-4*cq) + j.
    mtd_t = np.zeros((128, KC, 128), dtype=np.float32)
    for kc in range(KC):
        cq = kc // 4
        j0 = 128 * (kc - 4 * cq)
        qlo = 512 * cq + j0
        blk = m2[qlo:qlo + 128, 128 * kc:128 * (kc + 1)].T  # [sk 128, sq 128]
        mtd_t[:, kc, :] = (blk > -1e29)
    mtd_t = np.ascontiguousarray(mtd_t).astype(bf)

    wqf = np.asarray(wq, dtype=np.float32)
    wkf = np.asarray(wk, dtype=np.float32)
    wvf = np.asarray(wv, dtype=np.float32)
    wof = np.asarray(wo, dtype=np.float32)

    in_maps = []
    for c in range(N_CORES):
        wq_c = wqf[E * c:E * (c + 1), :]                    # [512, D]
        wqt = np.ascontiguousarray(
            wq_c.T.reshape(DC, 128, E).transpose(1, 0, 2)).astype(bf)
        wk_c = wkf[HD * c:HD * (c + 1), :]                  # [128, D]
        wv_c = wvf[HD * c:HD * (c + 1), :]
        wkv_c = np.concatenate([wk_c, wv_c], axis=0)        # [256, D]
        wkvt = np.ascontiguousarray(
            wkv_c.T.reshape(DC, 128, 2 * HD).transpose(1, 0, 2)).astype(bf)
        wo_c = wof[:, E * c:E * (c + 1)].T                  # [512 e, D]
        wo_t = np.ascontiguousarray(
            wo_c.reshape(HQ, 128, D).transpose(1, 0, 2)).astype(bf)
        in_maps.append({
            "xtt": xtt, "wqt": wqt, "wkvt": wkvt, "wo": wo_t,
            "mtd": mtd_t, "cos": cos_t, "sin": sin_t,
        })
    return in_maps


def kernel(x, wq, wk, wv, wo, freqs_cis, mask, start_pos=0):
    if "nc" not in _CACHE:
        _CACHE["nc"] = build_bass()
    nc = _CACHE["nc"]
    in_maps = _prep_inputs(x, wq, wk, wv, wo, freqs_cis, mask)
    res = run_bass_kernel_spmd(nc, in_maps, list(range(N_CORES)))
    total = np.zeros((S, D), dtype=np.float32)
    for c in range(N_CORES):
        total += res.results[c]["out"].astype(np.float32)
    return total.reshape(1, S, D)
